# revision 3
# baseline (speedup 1.0000x reference)
"""Trainium2 Bass kernel for DiscriminativeLossWHardNegatives, v2.

Same math as v1, but the gather uses batched DMAGatherAnt instructions
(1024 rows per instruction) instead of 256 serialized indirect DMAs,
removing the GpSimd descriptor-generation bottleneck (322us -> ~45us).

The gather requires int16 indices, so the host compacts the embedding
table per core: uniq = unique(batch_nns_core) (<= 32768 rows, fits int16
exactly), stages emb[uniq] in bf16, and remaps indices.

Sharding: data-parallel over batch across 8 cores (256 rows each).
"""

from contextlib import ExitStack

import numpy as np

import concourse.bass as bass
import concourse.bacc as bacc
import concourse.tile as tile
from concourse import mybir
from concourse import bass_utils

P = 128
N_CORES = 8
NGATH = 1024        # rows per dma_gather (ring limit: <= ~2032)
GB = NGATH // P     # neighbor block size per gather = 8


def emit_kernel(tc, ro_ap, idx_ap, emb_ap, loss_ap, acc_ap):
    """ro_ap: (B_pc, E) f32; idx_ap: (128, n_tiles*K/GB*NGATH//16) i16
    (wrapped+replicated); emb_ap: (32768, E) bf16 compacted;
    loss/acc: (B_pc,) f32."""
    nc = tc.nc
    B_pc, E = ro_ap.shape
    K = 128
    n_tiles = B_pc // P
    n_blocks = K // GB
    f32 = mybir.dt.float32
    bf16 = mybir.dt.bfloat16

    ctx = ExitStack()
    gath_pool = ctx.enter_context(tc.tile_pool(name="gath", bufs=4))
    io_pool = ctx.enter_context(tc.tile_pool(name="io", bufs=2))
    scr_pool = ctx.enter_context(tc.tile_pool(name="scr", bufs=3))
    col_pool = ctx.enter_context(tc.tile_pool(name="col", bufs=1))
    idx_pool = ctx.enter_context(tc.tile_pool(name="idx", bufs=1))


    idx_t = idx_pool.tile([P, idx_ap.shape[1]], mybir.dt.int16)
    nc.sync.dma_start(out=idx_t[:], in_=idx_ap[:, :])

    # ---- phase 1: norms + acc for ALL tiles (groups ACT table sets) ----
    ro_ts, ro_ds, invs = [], [], []
    for t in range(n_tiles):
        rows = slice(t * P, (t + 1) * P)
        ro_t = io_pool.tile([P, E], f32, tag=f"ro{t}")
        nc.sync.dma_start(out=ro_t[:], in_=ro_ap[rows, :])
        sq = scr_pool.tile([P, E], f32, tag="sqf")
        ss = col_pool.tile([P, 1], f32, tag=f"ss{t}")
        nc.scalar.activation(
            out=sq[:], in_=ro_t[:], func=mybir.ActivationFunctionType.Square,
            accum_out=ss[:],
        )
        nrm = col_pool.tile([P, 1], f32, tag=f"nrm{t}")
        nc.scalar.sqrt(nrm[:], ss[:])
        inv = col_pool.tile([P, 1], f32, tag=f"inv{t}", bufs=1)
        nc.vector.reciprocal(inv[:], nrm[:])
        rmax = col_pool.tile([P, 1], f32, tag=f"rmax{t}")
        nc.vector.reduce_max(out=rmax[:], in_=ro_t[:], axis=mybir.AxisListType.X)
        acc_col = col_pool.tile([P, 1], f32, tag=f"acc{t}")
        nc.vector.tensor_tensor(
            out=acc_col[:], in0=ro_t[:, 0:1], in1=rmax[:],
            op=mybir.AluOpType.is_ge,
        )
        nc.sync.dma_start(out=acc_ap[rows, None], in_=acc_col[:])
        ro_d = io_pool.tile([P, E], bf16, tag=f"rod{t}")
        nc.vector.tensor_copy(ro_d[:], ro_t[:])
        ro_ts.append(ro_t); ro_ds.append(ro_d); invs.append(inv)

    # ---- phase 2: gathers + dots + per-tile softmax tail ----
    for t in range(n_tiles):
        rows = slice(t * P, (t + 1) * P)
        ro_d, inv = ro_ds[t], invs[t]
        cos_t = io_pool.tile([P, K], f32, tag=f"cos{t}")
        for j in range(n_blocks):
            blk = t * n_blocks + j
            g = gath_pool.tile([P, GB, E], bf16)
            nc.gpsimd.dma_gather(
                out_ap=g[:], in_ap=emb_ap[:, :],
                idxs_ap=idx_t[:, blk * (NGATH // 16):(blk + 1) * (NGATH // 16)],
                num_idxs=NGATH, num_idxs_reg=NGATH,
                elem_size=E, transpose=False, queue_num=blk % 4,
            )
            for r in range(GB):
                n = j * GB + r
                if n % 10 < 3:
                    # DVE-only fused dot (1x): (g*inv)*ro with row accum.
                    prod = scr_pool.tile([P, E], f32, tag="sqf")
                    nc.vector.scalar_tensor_tensor(
                        out=prod[:], in0=g[:, r, :], scalar=inv[:], in1=ro_d[:],
                        op0=mybir.AluOpType.mult, op1=mybir.AluOpType.mult,
                        accum_out=cos_t[:, n:n + 1],
                    )
                else:
                    # DVE 2x mult; ACT scale-copy whose accumulator is the dot.
                    prod = scr_pool.tile([P, E], bf16, tag="prodb")
                    nc.vector.tensor_tensor(
                        out=prod[:], in0=g[:, r, :], in1=ro_d[:],
                        op=mybir.AluOpType.mult,
                    )
                    actout = scr_pool.tile([P, E], bf16, tag="actout")
                    nc.scalar.activation(
                        out=actout[:], in_=prod[:],
                        func=mybir.ActivationFunctionType.Copy,
                        bias=0.0, scale=inv[:],
                        accum_out=cos_t[:, n:n + 1],
                    )

        # ---- log-softmax tail: loss = log(sum(exp(cos-m))) - (cos0-m) ----
        mn = col_pool.tile([P, 1], f32)
        nc.vector.tensor_reduce(
            out=mn[:], in_=cos_t[:], axis=mybir.AxisListType.X,
            op=mybir.AluOpType.max, negate=True,
        )
        et = scr_pool.tile([P, K], f32)
        se = col_pool.tile([P, 1], f32)
        nc.scalar.activation(
            out=et[:], in_=cos_t[:], func=mybir.ActivationFunctionType.Exp,
            bias=mn[:], scale=1.0, accum_out=se[:],
        )
        ln = col_pool.tile([P, 1], f32)
        nc.scalar.activation(
            out=ln[:], in_=se[:], func=mybir.ActivationFunctionType.Ln,
        )
        t0 = col_pool.tile([P, 1], f32)
        nc.vector.tensor_tensor(
            out=t0[:], in0=cos_t[:, 0:1], in1=mn[:], op=mybir.AluOpType.add,
        )
        loss_col = col_pool.tile([P, 1], f32)
        nc.vector.tensor_tensor(
            out=loss_col[:], in0=ln[:], in1=t0[:], op=mybir.AluOpType.subtract,
        )
        nc.sync.dma_start(out=loss_ap[rows, None], in_=loss_col[:])

    ctx.close()


def build_nc(B_pc, E, K):
    nc = bacc.Bacc(
        "TRN2", target_bir_lowering=False, debug=False, enable_asserts=True,
        num_swdge_queues=4,
    )
    n_idx_cols = (B_pc // P) * (K // GB) * (NGATH // 16)
    ro = nc.dram_tensor("ro_in", (B_pc, E), mybir.dt.float32,
                        kind="ExternalInput").ap()
    idx = nc.dram_tensor("idx_in", (P, n_idx_cols), mybir.dt.int16,
                         kind="ExternalInput").ap()
    emb = nc.dram_tensor("emb_in", (32768, E), mybir.dt.bfloat16,
                         kind="ExternalInput").ap()
    loss = nc.dram_tensor("loss_out", (B_pc,), mybir.dt.float32,
                          kind="ExternalOutput").ap()
    acc = nc.dram_tensor("acc_out", (B_pc,), mybir.dt.float32,
                         kind="ExternalOutput").ap()
    with tile.TileContext(nc) as tc:
        emit_kernel(tc, ro, idx, emb, loss, acc)
    nc.compile()
    return nc


_NC_CACHE = {}


def _get_nc(B_pc, E, K):
    key = (B_pc, E, K)
    if key not in _NC_CACHE:
        _NC_CACHE[key] = build_nc(B_pc, E, K)
    return _NC_CACHE[key]


def _prep_core(batch_nns_core, embb):
    """Compact table + wrapped int16 idx layout for one core."""
    B_pc, K = batch_nns_core.shape
    flat = batch_nns_core.ravel()
    uniq, inverse = np.unique(flat, return_inverse=True)
    embc = np.zeros((32768, embb.shape[1]), dtype=embb.dtype)
    embc[: len(uniq)] = embb[uniq]
    inv2d = inverse.reshape(B_pc, K).astype(np.int16)
    n_tiles = B_pc // P
    n_blocks = K // GB
    cols = []
    for t in range(n_tiles):
        for j in range(n_blocks):
            # slot i of gather (t,j): (b_local=i%128, n_local=i//128)
            sub = inv2d[t * P:(t + 1) * P, j * GB:(j + 1) * GB]  # [128, GB]
            flat_i = sub.T.ravel()                               # i = n*128+b
            w16 = flat_i.reshape(NGATH // 16, 16).T              # [16, N/16]
            cols.append(np.tile(w16, (8, 1)))                    # [128, N/16]
    return embc, np.concatenate(cols, axis=1)


def _run(receiver_output, emb, nns, labels, num_hard_negatives, **spmd_kwargs):
    import ml_dtypes

    k = int(num_hard_negatives) + 1
    assert k == 128, "kernel specialized for K=128"
    ro = np.ascontiguousarray(np.asarray(receiver_output, dtype=np.float32))
    embb = np.asarray(emb).astype(ml_dtypes.bfloat16)
    B, E = ro.shape
    assert B % N_CORES == 0
    B_pc = B // N_CORES

    batch_nns = np.asarray(nns)[np.asarray(labels)][:, :k].astype(np.int64)

    nc = _get_nc(B_pc, E, k)
    in_maps = []
    for c in range(N_CORES):
        embc, idxw = _prep_core(batch_nns[c * B_pc:(c + 1) * B_pc], embb)
        in_maps.append({
            "ro_in": ro[c * B_pc:(c + 1) * B_pc],
            "idx_in": idxw,
            "emb_in": embc,
        })
    res = bass_utils.run_bass_kernel_spmd(
        nc, in_maps, core_ids=list(range(N_CORES)), **spmd_kwargs,
    )
    loss = np.concatenate([res.results[c]["loss_out"] for c in range(N_CORES)])
    acc = np.concatenate([res.results[c]["acc_out"] for c in range(N_CORES)])
    return (loss, acc), res


def kernel(receiver_output, emb, nns, labels, num_hard_negatives):
    out, _ = _run(receiver_output, emb, nns, labels, num_hard_negatives)
    return out


def _install_ntff_hook(so_path="/opt/axon/libaxon_pjrt.so"):
    """Provide antenv.axon_hooks (absent in this image) so that
    run_bass_kernel_spmd(trace=True) can capture NTFF profiles under axon.
    Mirrors trn_agent_boot/trn_boot.py's ctypes shim."""
    import contextlib
    import ctypes
    import os
    import sys
    import types

    if "antenv.axon_hooks" in sys.modules:
        return True
    if not os.path.exists(so_path):
        return False
    lib = ctypes.CDLL(so_path)
    if not hasattr(lib, "axon_start_nrt_profile"):
        return False
    lib.axon_start_nrt_profile.argtypes = [
        ctypes.POINTER(ctypes.c_int64),
        ctypes.c_size_t,
    ]
    lib.axon_start_nrt_profile.restype = ctypes.c_int64
    lib.axon_stop_nrt_profile.argtypes = [ctypes.c_char_p]
    lib.axon_stop_nrt_profile.restype = ctypes.c_int64

    @contextlib.contextmanager
    def _hook(output_dir, device_ids):
        import jax

        jax.devices()
        if device_ids:
            ids = (ctypes.c_int64 * len(device_ids))(*device_ids)
            rc = lib.axon_start_nrt_profile(ids, len(device_ids))
        else:
            rc = lib.axon_start_nrt_profile(None, 0)
        if rc != 0:
            raise RuntimeError(f"axon_start_nrt_profile rc={rc}")
        try:
            yield
        finally:
            n = lib.axon_stop_nrt_profile(str(output_dir).encode())
            print(f"ntff profile: {n} file(s) written to {output_dir}")

    box = {"hook": _hook}
    mod = types.ModuleType("antenv.axon_hooks")
    mod.get_axon_ntff_profile_hook = lambda: box["hook"]
    mod.set_axon_ntff_profile_hook = lambda h: box.__setitem__("hook", h)
    sys.modules["antenv.axon_hooks"] = mod
    return True


def kernel_profiled(receiver_output, emb, nns, labels, num_hard_negatives,
                    tmpdir=None):
    trace = _install_ntff_hook()
    return _run(
        receiver_output, emb, nns, labels, num_hard_negatives,
        trace=trace, tmpdir=tmpdir,
    )
